# revision 12
# baseline (speedup 1.0000x reference)
"""Trainium2 Bass kernel for nn_BayerUpsample4x4 — two-pass all-matmul design.

The reference op: x [4,1,1024,1024] -> 16-channel polyphase 4x bilinear
(tent-filter) upsample, output [4,16,1024,1024].  Channel k=(r,c) is x
subsampled at rows=r, cols=c (mod 4), zero-upsampled x4 and convolved
with the separable 7x7 tent kernel (zero padding at borders).

Kernel plan (8 cores = 4 batches x 2 row-halves; 512 out rows/core):
  The separable interp is two matmul contractions.  The host pre-builds a
  TRANSPOSED, phase-separated copy of the input so that BOTH passes
  contract over the partition dim (no on-chip transposes, no strided
  vector ops):

  pass 1 (horizontal):  Z_cr[v, q] = sum_t kh[q-(4t+c)] * x[4v+r, 4t+c]
      matmul: lhsT = xt tile [K=66 lattice cols, M = lattice rows],
              rhs  = banded const H_c [66, 256 out cols]
  pass 2 (vertical):    out[p, q] = sum_v kv[p-(4v+r)] * Z_cr[v, q]
      matmul: lhsT = banded const V_rb [K, M=128 out rows],
              rhs  = Z_cr [K, 512 out cols]

  All matmul operands bf16 (1 cyc/row), PSUM fp32.  PSUM evacuation on
  ScalarE+VectorE (dense, FD>=512).  One dense 2 MB DMA store per
  channel.  Everything overlaps behind the ~90us/core HBM store floor.

  Lattice rows per (r): ~130 -> an A-chunk (128 rows) serving out-blocks
  b=0..2 via band-embedded V (K=128), and a B-chunk (34 rows) serving
  b=3 (K=34).  Rows duplicated between chunks keep every pass-2 rhs
  window partition-contiguous from partition 0.
"""

import sys
for _p in ("/opt/trn_rl_repo", "/opt/pypackages"):
    if _p not in sys.path:
        sys.path.append(_p)

from contextlib import ExitStack

import numpy as np
import ml_dtypes

import concourse.bass as bass
import concourse.tile as tile
from concourse import bacc, mybir
from concourse.bass_utils import run_bass_kernel_spmd

F32 = mybir.dt.float32
BF16 = mybir.dt.bfloat16
AF = mybir.ActivationFunctionType

N_CORES = 8
H, W = 1024, 1024
HALF = 512                # output rows per core
KT = 66                   # pass-1 contraction (lattice-col window)
NA = 128                  # A-chunk lattice rows
NB = 34                   # B-chunk lattice rows
NV = 130                  # lattice rows per (r,c,j) window (A/B are slices)
CPRC = 4 * NV             # xt cols per (r,c)
XT_W = 16 * CPRC          # 8320

# (row, col) offset within each 4x4 block for channel k (matches reference)
OFFSETS = [(0, 0), (0, 2), (2, 0), (2, 2),
           (0, 1), (0, 3), (2, 1), (2, 3),
           (1, 0), (1, 2), (3, 0), (3, 2),
           (1, 1), (1, 3), (3, 1), (3, 3)]
K_OF = {rc: k for k, rc in enumerate(OFFSETS)}


def _ceil_div(a, b):
    return -((-a) // b)


def _v_lo(P0, r):
    return _ceil_div(P0 - 3 - r, 4)


def _emit(tc, xs, hm, vm, out):
    """Trace the per-core program.

    xs:  [66, 8320] bf16 packed pass-1 lhsT tiles (host-built)
    hm:  [66, 1024]  bf16 horizontal interp matrices (4x [66,256])
    vm:  [128, 2048] bf16 vertical interp matrices (16x [128,128], (r,b))
    out: [16, 512, 1024] f32
    """
    nc = tc.nc

    with ExitStack() as ctx:
        cpool = ctx.enter_context(tc.tile_pool(name="const", bufs=2))
        xpool = ctx.enter_context(tc.tile_pool(name="xtp", bufs=2))
        pApool = ctx.enter_context(tc.tile_pool(name="psA", bufs=1,
                                                space="PSUM"))
        pBpool = ctx.enter_context(tc.tile_pool(name="psB", bufs=1,
                                                space="PSUM"))
        pOpool = ctx.enter_context(tc.tile_pool(name="psO", bufs=3,
                                                space="PSUM"))
        zpool = ctx.enter_context(tc.tile_pool(name="z", bufs=3))
        opool = ctx.enter_context(tc.tile_pool(name="o", bufs=3))

        hmt = cpool.tile([KT, 1024], BF16, tag="hm")
        nc.scalar.dma_start(hmt[:], hm)
        vmt = cpool.tile([128, 16 * 128], BF16, tag="vm")
        nc.scalar.dma_start(vmt[:], vm)

        xs_r = xs.rearrange("t (r w) -> r t w", r=4)

        evac_flip = 0
        for r in range(4):
            xt = xpool.tile([KT, 4 * CPRC], BF16, tag="xtr")
            nc.scalar.dma_start(xt[:], xs_r[r])
            for c in range(4):
                k = K_OF[(r, c)]
                base = c * CPRC
                Hc = hmt[:, 256 * c: 256 * c + 256]

                psA = pApool.tile([NA, 1024], F32, tag="psA")
                psB = pBpool.tile([NB, 1024], F32, tag="psB")
                for j in range(4):
                    w0 = base + NV * j
                    nc.tensor.matmul(
                        psA[:, 256 * j: 256 * j + 256],
                        lhsT=xt[:, w0: w0 + NA],
                        rhs=Hc, start=True, stop=True)
                    nc.tensor.matmul(
                        psB[:, 256 * j: 256 * j + 256],
                        lhsT=xt[:, w0 + 96: w0 + NV],
                        rhs=Hc, start=True, stop=True)

                zA = zpool.tile([NA, 1024], BF16, tag="zA")
                zB = zpool.tile([NB, 1024], BF16, tag="zB")
                nc.scalar.activation(zA[:, 0:512], psA[:, 0:512], AF.Copy)
                nc.vector.tensor_copy(zA[:, 512:1024], psA[:, 512:1024])
                nc.scalar.activation(zB[:], psB[:], AF.Copy)

                ot = opool.tile([128, 4096], F32, tag="ot")
                for b in range(4):
                    vblk = vmt[:, (r * 4 + b) * 128: (r * 4 + b) * 128 + 128]
                    if b < 3:
                        lhsT, rsrc = vblk, zA
                    else:
                        lhsT, rsrc = vblk[0:NB, :], zB
                    for nchk in range(2):
                        psO = pOpool.tile([128, 512], F32, tag="psO")
                        nc.tensor.matmul(
                            psO[:], lhsT=lhsT,
                            rhs=rsrc[:, 512 * nchk: 512 * nchk + 512],
                            start=True, stop=True)
                        dst = ot[:, 1024 * b + 512 * nchk:
                                 1024 * b + 512 * nchk + 512]
                        if evac_flip % 2 == 0:
                            nc.scalar.activation(dst, psO[:], AF.Copy)
                        else:
                            nc.vector.tensor_copy(dst, psO[:])
                        evac_flip += 1

                nc.sync.dma_start(
                    out[k].rearrange("(b p) q -> p b q", b=4),
                    ot[:].rearrange("p (b q) -> p b q", q=1024))


_CACHE = {}


def _build_module():
    if "m" in _CACHE:
        return _CACHE["m"]
    nc = bacc.Bacc("TRN2", target_bir_lowering=False, debug=False)
    xs = nc.dram_tensor("xs", [KT, XT_W], BF16, kind="ExternalInput").ap()
    hm = nc.dram_tensor("hm", [KT, 1024], BF16, kind="ExternalInput").ap()
    vm = nc.dram_tensor("vm", [128, 16 * 128], BF16,
                        kind="ExternalInput").ap()
    out = nc.dram_tensor("out", [16, HALF, W], F32,
                         kind="ExternalOutput").ap()
    with tile.TileContext(nc) as tc:
        _emit(tc, xs, hm, vm, out)
    nc.compile()
    _CACHE["m"] = nc
    return nc


def _hmat(kh):
    """[66, 1024] f32: 4 horizontal interp blocks H_c [66, 256].

    H_c[t', q'] = kh[7 + q' - 4t' - c] where in [0,7); out col q = 256j+q'
    reads lattice col 4(64j-1+t') + c (zero-padded xt rows handle borders).
    """
    hm = np.zeros((KT, 1024), np.float32)
    tp = np.arange(KT)
    qp = np.arange(256)
    for c in range(4):
        e = 7 + qp[None, :] - 4 * tp[:, None] - c
        m = (e >= 0) & (e <= 6)
        hm[:, 256 * c: 256 * c + 256][m] = kh[e[m]]
    return hm


def _vmat(kv, half):
    """[128, 2048] f32: 16 vertical blocks V_(r,b) [128, 128].

    b<3: band-embedded, V[s, m] = kv[3 + (P0+128b+m) - (4(v_lo+s)+r)].
    b=3: rows 0..33 for the B-chunk (v = v_lo+96+s), rest zero.
    """
    P0 = 512 * half
    vm = np.zeros((128, 16 * 128), np.float32)
    mm = np.arange(128)
    for r in range(4):
        vlo = _v_lo(P0, r)
        for b in range(4):
            s = np.arange(128 if b < 3 else NB)
            vbase = vlo if b < 3 else vlo + 96
            d = 3 + (P0 + 128 * b + mm[None, :]) \
                - (4 * (vbase + s[:, None]) + r)
            msk = (d >= 0) & (d <= 6)
            blk = np.zeros((128, 128), np.float32)
            sub = np.zeros(d.shape, np.float32)
            sub[msk] = kv[d[msk]]
            blk[: d.shape[0]] = sub
            vm[:, (r * 4 + b) * 128: (r * 4 + b) * 128 + 128] = blk
    return vm


def _xt_core(xp, half):
    """[66, 8320] f32 packed pass-1 lhsT tiles for one (image, half).

    xp: [1040, 1032] zero-padded image, xp[4+i, 4+j] = x[i, j].
    Window (r,c,j)[t', m] = x[4(v_lo+m)+r, 256j - 4 + 4t' + c], m in
    [0,130); the A-chunk lhsT is cols [0,128), B-chunk cols [96,130).
    """
    P0 = 512 * half
    xt = np.zeros((KT, XT_W), np.float32)
    tp4 = 4 * np.arange(KT)
    for r in range(4):
        vlo = _v_lo(P0, r)
        rows = 4 * (vlo + np.arange(NV)) + r + 4
        for c in range(4):
            base = (r * 4 + c) * CPRC
            for j in range(4):
                cols = 256 * j + tp4 + c          # +4 pad -4 offset
                xt[:, base + NV * j: base + NV * j + NV] = \
                    xp[np.ix_(rows, cols)].T
    return xt


def _host_inputs(x, weight):
    x = np.asarray(x, np.float32)
    weight = np.asarray(weight, np.float32)
    k2 = weight[0, 0]
    kv = k2[:, 3].astype(np.float32)
    kh = k2[3, :].astype(np.float32)

    bf = ml_dtypes.bfloat16
    hm = _hmat(kh).astype(bf)
    vms = [_vmat(kv, h).astype(bf) for h in range(2)]

    in_maps = []
    for core in range(N_CORES):
        n, half = divmod(core, 2)
        xp = np.zeros((H + 16, W + 8), np.float32)
        xp[4:4 + H, 4:4 + W] = x[n, 0]
        in_maps.append({"xs": _xt_core(xp, half).astype(bf),
                        "hm": hm, "vm": vms[half]})
    return in_maps


def kernel(x, weight):
    assert np.asarray(x).shape == (4, 1, H, W)
    nc = _build_module()
    in_maps = _host_inputs(x, weight)
    res = run_bass_kernel_spmd(nc, in_maps, list(range(N_CORES)))

    full = np.empty((4, 16, H, W), np.float32)
    for core in range(N_CORES):
        n, half = divmod(core, 2)
        full[n, :, 512 * half: 512 * half + 512, :] = \
            res.results[core]["out"]
    return full


# revision 16
# speedup vs baseline: 1.1150x; 1.1150x over previous
"""Trainium2 Bass kernel for nn_BayerUpsample4x4 — two-pass all-matmul design.

The reference op: x [4,1,1024,1024] -> 16-channel polyphase 4x bilinear
(tent-filter) upsample, output [4,16,1024,1024].  Channel k=(r,c) is x
subsampled at rows=r, cols=c (mod 4), zero-upsampled x4 and convolved
with the separable 7x7 tent kernel (zero padding at borders).

Kernel plan (8 cores = 4 batches x 2 row-halves; 512 out rows/core):
  The separable interp is two matmul contractions.  The host pre-builds a
  TRANSPOSED, phase-separated copy of the input so that BOTH passes
  contract over the partition dim (no on-chip transposes, no strided
  vector ops):

  pass 1 (horizontal):  Z_cr[v, q] = sum_t kh[q-(4t+c)] * x[4v+r, 4t+c]
      matmul: lhsT = xt tile [K=66 lattice cols, M = lattice rows],
              rhs  = banded const H_c [66, 256 out cols]
  pass 2 (vertical):    out[p, q] = sum_v kv[p-(4v+r)] * Z_cr[v, q]
      matmul: lhsT = banded const V_rb [K, M=128 out rows],
              rhs  = Z_cr [K, 512 out cols]

  All matmul operands bf16 (1 cyc/row), PSUM fp32.  PSUM evacuation on
  ScalarE+VectorE (dense, FD>=512).  One dense 2 MB DMA store per
  channel.  Everything overlaps behind the ~90us/core HBM store floor.

  Lattice rows per (r): ~130 -> an A-chunk (128 rows) serving out-blocks
  b=0..2 via band-embedded V (K=128), and a B-chunk (34 rows) serving
  b=3 (K=34).  Rows duplicated between chunks keep every pass-2 rhs
  window partition-contiguous from partition 0.
"""

import sys
for _p in ("/opt/trn_rl_repo", "/opt/pypackages"):
    if _p not in sys.path:
        sys.path.append(_p)

from contextlib import ExitStack

import numpy as np
import ml_dtypes

import concourse.bass as bass
import concourse.tile as tile
from concourse import bacc, mybir
from concourse.bass_utils import run_bass_kernel_spmd

F32 = mybir.dt.float32
BF16 = mybir.dt.bfloat16
AF = mybir.ActivationFunctionType

N_CORES = 8
H, W = 1024, 1024
HALF = 512                # output rows per core
KT = 66                   # pass-1 contraction (lattice-col window)
NA = 128                  # A-chunk lattice rows
NB = 34                   # B-chunk lattice rows
NV = 130                  # lattice rows per (r,c,j) window (A/B are slices)
CPRC = 4 * NV             # xt cols per (r,c)
XT_W = 16 * CPRC          # 8320

# (row, col) offset within each 4x4 block for channel k (matches reference)
OFFSETS = [(0, 0), (0, 2), (2, 0), (2, 2),
           (0, 1), (0, 3), (2, 1), (2, 3),
           (1, 0), (1, 2), (3, 0), (3, 2),
           (1, 1), (1, 3), (3, 1), (3, 3)]
K_OF = {rc: k for k, rc in enumerate(OFFSETS)}


def _ceil_div(a, b):
    return -((-a) // b)


def _v_lo(P0, r):
    return _ceil_div(P0 - 3 - r, 4)


def _emit(tc, xs, hm, vm, out, n_rep=1):
    """Trace the per-core program.

    xs:  [66, 8320] bf16 packed pass-1 lhsT tiles (host-built)
    hm:  [66, 1024]  bf16 horizontal interp matrices (4x [66,256])
    vm:  [128, 2048] bf16 vertical interp matrices (16x [128,128], (r,b))
    out: [16, 512, 1024] f32
    """
    nc = tc.nc

    with ExitStack() as ctx:
        cpool = ctx.enter_context(tc.tile_pool(name="const", bufs=2))
        xpool = ctx.enter_context(tc.tile_pool(name="xtp", bufs=2))
        pApool = ctx.enter_context(tc.tile_pool(name="psA", bufs=1,
                                                space="PSUM"))
        pBpool = ctx.enter_context(tc.tile_pool(name="psB", bufs=1,
                                                space="PSUM"))
        pOpool = ctx.enter_context(tc.tile_pool(name="psO", bufs=3,
                                                space="PSUM"))
        zpool = ctx.enter_context(tc.tile_pool(name="z", bufs=3))
        opool = ctx.enter_context(tc.tile_pool(name="o", bufs=3))

        evac_flip = 0
        for _rep in range(n_rep):
          hmt = cpool.tile([KT, 1024], BF16, tag="hm")
          nc.sync.dma_start(hmt[:], hm)
          vmt = cpool.tile([128, 16 * 128], BF16, tag="vm")
          nc.sync.dma_start(vmt[:], vm)
          xt = xpool.tile([KT, XT_W], BF16, tag="xtr")
          nc.sync.dma_start(xt[:], xs)
          for r in range(4):
            for c in range(4):
                k = K_OF[(r, c)]
                base = (r * 4 + c) * CPRC
                Hc = hmt[:, 256 * c: 256 * c + 256]

                psA = pApool.tile([NA, 1024], F32, tag="psA")
                psB = pBpool.tile([NB, 1024], F32, tag="psB")
                for j in range(4):
                    w0 = base + NV * j
                    nc.tensor.matmul(
                        psA[:, 256 * j: 256 * j + 256],
                        lhsT=xt[:, w0: w0 + NA],
                        rhs=Hc, start=True, stop=True)
                    nc.tensor.matmul(
                        psB[:, 256 * j: 256 * j + 256],
                        lhsT=xt[:, w0 + 96: w0 + NV],
                        rhs=Hc, start=True, stop=True)

                zA = zpool.tile([NA, 1024], BF16, tag="zA")
                zB = zpool.tile([NB, 1024], BF16, tag="zB")
                nc.scalar.activation(zA[:, 0:512], psA[:, 0:512], AF.Copy)
                nc.vector.tensor_copy(zA[:, 512:1024], psA[:, 512:1024])
                nc.scalar.activation(zB[:], psB[:], AF.Copy)

                ot = opool.tile([128, 4096], F32, tag="ot")
                for b in range(4):
                    vblk = vmt[:, (r * 4 + b) * 128: (r * 4 + b) * 128 + 128]
                    if b < 3:
                        lhsT, rsrc = vblk, zA
                    else:
                        lhsT, rsrc = vblk[0:NB, :], zB
                    for nchk in range(2):
                        psO = pOpool.tile([128, 512], F32, tag="psO")
                        nc.tensor.matmul(
                            psO[:], lhsT=lhsT,
                            rhs=rsrc[:, 512 * nchk: 512 * nchk + 512],
                            start=True, stop=True)
                        dst = ot[:, 1024 * b + 512 * nchk:
                                 1024 * b + 512 * nchk + 512]
                        if evac_flip % 2 == 0:
                            nc.scalar.activation(dst, psO[:], AF.Copy)
                        else:
                            nc.vector.tensor_copy(dst, psO[:])
                        evac_flip += 1

                nc.sync.dma_start(
                    out[k].rearrange("(b p) q -> p b q", b=4),
                    ot[:].rearrange("p (b q) -> p b q", q=1024))


_CACHE = {}


def _build_module():
    if "m" in _CACHE:
        return _CACHE["m"]
    nc = bacc.Bacc("TRN2", target_bir_lowering=False, debug=False)
    xs = nc.dram_tensor("xs", [KT, XT_W], BF16, kind="ExternalInput").ap()
    hm = nc.dram_tensor("hm", [KT, 1024], BF16, kind="ExternalInput").ap()
    vm = nc.dram_tensor("vm", [128, 16 * 128], BF16,
                        kind="ExternalInput").ap()
    out = nc.dram_tensor("out", [16, HALF, W], F32,
                         kind="ExternalOutput").ap()
    with tile.TileContext(nc) as tc:
        _emit(tc, xs, hm, vm, out)
    nc.compile()
    _CACHE["m"] = nc
    return nc


def _hmat(kh):
    """[66, 1024] f32: 4 horizontal interp blocks H_c [66, 256].

    H_c[t', q'] = kh[7 + q' - 4t' - c] where in [0,7); out col q = 256j+q'
    reads lattice col 4(64j-1+t') + c (zero-padded xt rows handle borders).
    """
    hm = np.zeros((KT, 1024), np.float32)
    tp = np.arange(KT)
    qp = np.arange(256)
    for c in range(4):
        e = 7 + qp[None, :] - 4 * tp[:, None] - c
        m = (e >= 0) & (e <= 6)
        hm[:, 256 * c: 256 * c + 256][m] = kh[e[m]]
    return hm


def _vmat(kv, half):
    """[128, 2048] f32: 16 vertical blocks V_(r,b) [128, 128].

    b<3: band-embedded, V[s, m] = kv[3 + (P0+128b+m) - (4(v_lo+s)+r)].
    b=3: rows 0..33 for the B-chunk (v = v_lo+96+s), rest zero.
    """
    P0 = 512 * half
    vm = np.zeros((128, 16 * 128), np.float32)
    mm = np.arange(128)
    for r in range(4):
        vlo = _v_lo(P0, r)
        for b in range(4):
            s = np.arange(128 if b < 3 else NB)
            vbase = vlo if b < 3 else vlo + 96
            d = 3 + (P0 + 128 * b + mm[None, :]) \
                - (4 * (vbase + s[:, None]) + r)
            msk = (d >= 0) & (d <= 6)
            blk = np.zeros((128, 128), np.float32)
            sub = np.zeros(d.shape, np.float32)
            sub[msk] = kv[d[msk]]
            blk[: d.shape[0]] = sub
            vm[:, (r * 4 + b) * 128: (r * 4 + b) * 128 + 128] = blk
    return vm


def _xt_core(xp, half):
    """[66, 8320] f32 packed pass-1 lhsT tiles for one (image, half).

    xp: [1040, 1032] zero-padded image, xp[4+i, 4+j] = x[i, j].
    Window (r,c,j)[t', m] = x[4(v_lo+m)+r, 256j - 4 + 4t' + c], m in
    [0,130); the A-chunk lhsT is cols [0,128), B-chunk cols [96,130).
    """
    P0 = 512 * half
    xt = np.zeros((KT, XT_W), np.float32)
    tp4 = 4 * np.arange(KT)
    for r in range(4):
        vlo = _v_lo(P0, r)
        rows = 4 * (vlo + np.arange(NV)) + r + 4
        for c in range(4):
            base = (r * 4 + c) * CPRC
            for j in range(4):
                cols = 256 * j + tp4 + c          # +4 pad -4 offset
                xt[:, base + NV * j: base + NV * j + NV] = \
                    xp[np.ix_(rows, cols)].T
    return xt


def _host_inputs(x, weight):
    x = np.asarray(x, np.float32)
    weight = np.asarray(weight, np.float32)
    k2 = weight[0, 0]
    kv = k2[:, 3].astype(np.float32)
    kh = k2[3, :].astype(np.float32)

    bf = ml_dtypes.bfloat16
    hm = _hmat(kh).astype(bf)
    vms = [_vmat(kv, h).astype(bf) for h in range(2)]

    in_maps = []
    for core in range(N_CORES):
        n, half = divmod(core, 2)
        xp = np.zeros((H + 16, W + 8), np.float32)
        xp[4:4 + H, 4:4 + W] = x[n, 0]
        in_maps.append({"xs": _xt_core(xp, half).astype(bf),
                        "hm": hm, "vm": vms[half]})
    return in_maps


def kernel(x, weight):
    assert np.asarray(x).shape == (4, 1, H, W)
    nc = _build_module()
    in_maps = _host_inputs(x, weight)
    res = run_bass_kernel_spmd(nc, in_maps, list(range(N_CORES)))

    full = np.empty((4, 16, H, W), np.float32)
    for core in range(N_CORES):
        n, half = divmod(core, 2)
        full[n, :, 512 * half: 512 * half + 512, :] = \
            res.results[core]["out"]
    return full


# revision 19
# speedup vs baseline: 1.3393x; 1.2012x over previous
"""Trainium2 Bass kernel for nn_BayerUpsample4x4 — two-pass all-matmul design.

The reference op: x [4,1,1024,1024] -> 16-channel polyphase 4x bilinear
(tent-filter) upsample, output [4,16,1024,1024].  Channel k=(r,c) is x
subsampled at rows=r, cols=c (mod 4), zero-upsampled x4 and convolved
with the separable 7x7 tent kernel (zero padding at borders).

Kernel plan (8 cores = 4 batches x 2 row-halves; 512 out rows/core):
  The separable interp is two matmul contractions.  The host pre-builds a
  TRANSPOSED, phase-separated copy of the input so that BOTH passes
  contract over the partition dim (no on-chip transposes, no strided
  vector ops):

  pass 1 (horizontal):  Z_cr[v, q] = sum_t kh[q-(4t+c)] * x[4v+r, 4t+c]
      matmul: lhsT = xt tile [K=66 lattice cols, M = lattice rows],
              rhs  = banded const H_c [66, 256 out cols]
  pass 2 (vertical):    out[p, q] = sum_v kv[p-(4v+r)] * Z_cr[v, q]
      matmul: lhsT = banded const V_rb [K, M=128 out rows],
              rhs  = Z_cr [K, 512 out cols]

  All matmul operands bf16 (1 cyc/row), PSUM fp32.  PSUM evacuation on
  ScalarE+VectorE (dense, FD>=512).  One dense 2 MB DMA store per
  channel.  Everything overlaps behind the ~90us/core HBM store floor.

  Lattice rows per (r): ~130 -> an A-chunk (128 rows) serving out-blocks
  b=0..2 via band-embedded V (K=128), and a B-chunk (34 rows) serving
  b=3 (K=34).  Rows duplicated between chunks keep every pass-2 rhs
  window partition-contiguous from partition 0.
"""

import sys
for _p in ("/opt/trn_rl_repo", "/opt/pypackages"):
    if _p not in sys.path:
        sys.path.append(_p)

from contextlib import ExitStack

import numpy as np
import ml_dtypes

import concourse.bass as bass
import concourse.tile as tile
from concourse import bacc, mybir
from concourse.bass_utils import run_bass_kernel_spmd

F32 = mybir.dt.float32
BF16 = mybir.dt.bfloat16
AF = mybir.ActivationFunctionType

N_CORES = 8
H, W = 1024, 1024
HALF = 512                # output rows per core
KT = 66                   # pass-1 contraction (lattice-col window)
NA = 128                  # A-chunk lattice rows
NB = 34                   # B-chunk lattice rows
NV = 130                  # lattice rows per (r,c,j) window (A/B are slices)
CPRC = 4 * NV             # xt cols per (r,c)
XT_W = 16 * CPRC          # 8320

# (row, col) offset within each 4x4 block for channel k (matches reference)
OFFSETS = [(0, 0), (0, 2), (2, 0), (2, 2),
           (0, 1), (0, 3), (2, 1), (2, 3),
           (1, 0), (1, 2), (3, 0), (3, 2),
           (1, 1), (1, 3), (3, 1), (3, 3)]
K_OF = {rc: k for k, rc in enumerate(OFFSETS)}


def _ceil_div(a, b):
    return -((-a) // b)


def _v_lo(P0, r):
    return _ceil_div(P0 - 3 - r, 4)


CARRY = 2         # channel stores deferred across the loop-iteration barrier


def _emit(tc, xs, hm, vm, out, n_iter=1, n_rep=1, carry=CARRY):
    """Trace the per-core program (owns the For_i loop when n_iter > 1).

    xs:  [66, 8320] bf16 packed pass-1 lhsT tiles (host-built)
    hm:  [66, 1024]  bf16 horizontal interp matrices (4x [66,256])
    vm:  [128, 2048] bf16 vertical interp matrices (16x [128,128], (r,b))
    out: [16, 512, 1024] f32

    The last `carry` channels' stores are deferred: in loop mode they are
    issued at the TOP of the next iteration (the For_i all-engine barrier
    otherwise exposes the load+first-compute head with an idle DMA), and
    flushed after the loop.
    """
    nc = tc.nc

    with ExitStack() as ctx:
        cpool = ctx.enter_context(tc.tile_pool(name="const", bufs=1))
        xpool = ctx.enter_context(tc.tile_pool(name="xtp", bufs=1))
        pApool = ctx.enter_context(tc.tile_pool(name="psA", bufs=1,
                                                space="PSUM"))
        pBpool = ctx.enter_context(tc.tile_pool(name="psB", bufs=1,
                                                space="PSUM"))
        pOpool = ctx.enter_context(tc.tile_pool(name="psO", bufs=3,
                                                space="PSUM"))
        zpool = ctx.enter_context(tc.tile_pool(name="z", bufs=2))
        opool = ctx.enter_context(tc.tile_pool(name="o", bufs=2))
        ocpool = ctx.enter_context(tc.tile_pool(name="oc", bufs=1))

        steps = [(r, c) for r in range(4) for c in range(4)]
        n_steps = len(steps)

        def store(ot, k):
            nc.sync.dma_start(
                out[k].rearrange("(b p) q -> p b q", b=4),
                ot[:].rearrange("p (b q) -> p b q", q=1024))

        def body(defer):
            """One full iteration. Returns deferred [(ot, k)] to store."""
            oc = []
            if defer:
                for i in range(carry):
                    oc_i = ocpool.tile([128, 4096], F32, tag=f"oc{i}")
                    oc.append(oc_i)
            hmt = cpool.tile([KT, 1024], BF16, tag="hm")
            nc.sync.dma_start(hmt[:], hm)
            vmt = cpool.tile([128, 16 * 128], BF16, tag="vm")
            nc.sync.dma_start(vmt[:], vm)
            xt = xpool.tile([KT, XT_W], BF16, tag="xtr")
            nc.sync.dma_start(xt[:], xs)
            if defer:
                # previous iteration's data (garbage on iter 0 -> dump only)
                for i, (r, c) in enumerate(steps[n_steps - carry:]):
                    store(oc[i], K_OF[(r, c)])

            evac_flip = 0
            deferred = []
            for s, (r, c) in enumerate(steps):
                k = K_OF[(r, c)]
                base = (r * 4 + c) * CPRC
                Hc = hmt[:, 256 * c: 256 * c + 256]

                psA = pApool.tile([NA, 1024], F32, tag="psA")
                psB = pBpool.tile([NB, 1024], F32, tag="psB")
                for j in range(4):
                    w0 = base + NV * j
                    nc.tensor.matmul(
                        psA[:, 256 * j: 256 * j + 256],
                        lhsT=xt[:, w0: w0 + NA],
                        rhs=Hc, start=True, stop=True)
                    nc.tensor.matmul(
                        psB[:, 256 * j: 256 * j + 256],
                        lhsT=xt[:, w0 + 96: w0 + NV],
                        rhs=Hc, start=True, stop=True)

                zA = zpool.tile([NA, 1024], BF16, tag="zA")
                zB = zpool.tile([NB, 1024], BF16, tag="zB")
                nc.scalar.activation(zA[:, 0:512], psA[:, 0:512], AF.Copy)
                nc.vector.tensor_copy(zA[:, 512:1024], psA[:, 512:1024])
                nc.scalar.activation(zB[:], psB[:], AF.Copy)

                late = defer and s >= n_steps - carry
                if late:
                    ot = oc[s - (n_steps - carry)]
                else:
                    ot = opool.tile([128, 4096], F32, tag="ot")
                for b in range(4):
                    vblk = vmt[:, (r * 4 + b) * 128: (r * 4 + b) * 128 + 128]
                    if b < 3:
                        lhsT, rsrc = vblk, zA
                    else:
                        lhsT, rsrc = vblk[0:NB, :], zB
                    for nchk in range(2):
                        psO = pOpool.tile([128, 512], F32, tag="psO")
                        nc.tensor.matmul(
                            psO[:], lhsT=lhsT,
                            rhs=rsrc[:, 512 * nchk: 512 * nchk + 512],
                            start=True, stop=True)
                        dst = ot[:, 1024 * b + 512 * nchk:
                                 1024 * b + 512 * nchk + 512]
                        if evac_flip % 2 == 0:
                            nc.scalar.activation(dst, psO[:], AF.Copy)
                        else:
                            nc.vector.tensor_copy(dst, psO[:])
                        evac_flip += 1

                if late:
                    deferred.append((ot, k))
                else:
                    store(ot, k)
            return deferred

        if n_iter > 1:
            with tc.For_i(0, n_iter, 1):
                deferred = body(defer=True)
            for ot, k in deferred:
                store(ot, k)
        else:
            for _rep in range(n_rep):
                deferred = body(defer=(n_rep > 1))
                if n_rep == 1:
                    pass  # stores were inline (defer False)
            if n_rep > 1:
                for ot, k in deferred:
                    store(ot, k)


_CACHE = {}


def _build_module():
    if "m" in _CACHE:
        return _CACHE["m"]
    nc = bacc.Bacc("TRN2", target_bir_lowering=False, debug=False)
    xs = nc.dram_tensor("xs", [KT, XT_W], BF16, kind="ExternalInput").ap()
    hm = nc.dram_tensor("hm", [KT, 1024], BF16, kind="ExternalInput").ap()
    vm = nc.dram_tensor("vm", [128, 16 * 128], BF16,
                        kind="ExternalInput").ap()
    out = nc.dram_tensor("out", [16, HALF, W], F32,
                         kind="ExternalOutput").ap()
    with tile.TileContext(nc) as tc:
        _emit(tc, xs, hm, vm, out)
    nc.compile()
    _CACHE["m"] = nc
    return nc


def _hmat(kh):
    """[66, 1024] f32: 4 horizontal interp blocks H_c [66, 256].

    H_c[t', q'] = kh[7 + q' - 4t' - c] where in [0,7); out col q = 256j+q'
    reads lattice col 4(64j-1+t') + c (zero-padded xt rows handle borders).
    """
    hm = np.zeros((KT, 1024), np.float32)
    tp = np.arange(KT)
    qp = np.arange(256)
    for c in range(4):
        e = 7 + qp[None, :] - 4 * tp[:, None] - c
        m = (e >= 0) & (e <= 6)
        hm[:, 256 * c: 256 * c + 256][m] = kh[e[m]]
    return hm


def _vmat(kv, half):
    """[128, 2048] f32: 16 vertical blocks V_(r,b) [128, 128].

    b<3: band-embedded, V[s, m] = kv[3 + (P0+128b+m) - (4(v_lo+s)+r)].
    b=3: rows 0..33 for the B-chunk (v = v_lo+96+s), rest zero.
    """
    P0 = 512 * half
    vm = np.zeros((128, 16 * 128), np.float32)
    mm = np.arange(128)
    for r in range(4):
        vlo = _v_lo(P0, r)
        for b in range(4):
            s = np.arange(128 if b < 3 else NB)
            vbase = vlo if b < 3 else vlo + 96
            d = 3 + (P0 + 128 * b + mm[None, :]) \
                - (4 * (vbase + s[:, None]) + r)
            msk = (d >= 0) & (d <= 6)
            blk = np.zeros((128, 128), np.float32)
            sub = np.zeros(d.shape, np.float32)
            sub[msk] = kv[d[msk]]
            blk[: d.shape[0]] = sub
            vm[:, (r * 4 + b) * 128: (r * 4 + b) * 128 + 128] = blk
    return vm


def _xt_core(xp, half):
    """[66, 8320] f32 packed pass-1 lhsT tiles for one (image, half).

    xp: [1040, 1032] zero-padded image, xp[4+i, 4+j] = x[i, j].
    Window (r,c,j)[t', m] = x[4(v_lo+m)+r, 256j - 4 + 4t' + c], m in
    [0,130); the A-chunk lhsT is cols [0,128), B-chunk cols [96,130).
    """
    P0 = 512 * half
    xt = np.zeros((KT, XT_W), np.float32)
    tp4 = 4 * np.arange(KT)
    for r in range(4):
        vlo = _v_lo(P0, r)
        rows = 4 * (vlo + np.arange(NV)) + r + 4
        for c in range(4):
            base = (r * 4 + c) * CPRC
            for j in range(4):
                cols = 256 * j + tp4 + c          # +4 pad -4 offset
                xt[:, base + NV * j: base + NV * j + NV] = \
                    xp[np.ix_(rows, cols)].T
    return xt


def _host_inputs(x, weight):
    x = np.asarray(x, np.float32)
    weight = np.asarray(weight, np.float32)
    k2 = weight[0, 0]
    kv = k2[:, 3].astype(np.float32)
    kh = k2[3, :].astype(np.float32)

    bf = ml_dtypes.bfloat16
    hm = _hmat(kh).astype(bf)
    vms = [_vmat(kv, h).astype(bf) for h in range(2)]

    in_maps = []
    for core in range(N_CORES):
        n, half = divmod(core, 2)
        xp = np.zeros((H + 16, W + 8), np.float32)
        xp[4:4 + H, 4:4 + W] = x[n, 0]
        in_maps.append({"xs": _xt_core(xp, half).astype(bf),
                        "hm": hm, "vm": vms[half]})
    return in_maps


def kernel(x, weight):
    assert np.asarray(x).shape == (4, 1, H, W)
    nc = _build_module()
    in_maps = _host_inputs(x, weight)
    res = run_bass_kernel_spmd(nc, in_maps, list(range(N_CORES)))

    full = np.empty((4, 16, H, W), np.float32)
    for core in range(N_CORES):
        n, half = divmod(core, 2)
        full[n, :, 512 * half: 512 * half + 512, :] = \
            res.results[core]["out"]
    return full
